# revision 4
# baseline (speedup 1.0000x reference)
"""Causal self-attention with relative position encoding on 8 Trainium2 NeuronCores.

Problem: B=4, T=1024, C=256, H=8, E=32.
  q,k,v = x@W{q,k,v}.T ; att = q·k + einsum('qjhe,bhqe->bhqj', rel, k) ; scaled,
  causal-masked softmax ; y = att@v ; out = y@Wo.T

Sharding: query-row interleave across 8 cores (core m owns q = m+8t, t in [0,128)).
Each core loads a causally-trimmed, host-transposed pack of rel_encoding (~68 MiB
instead of 1 GiB), computes its 128 output rows fully, and the host re-interleaves.

Device-side structure per core (see block comments below):
 - scores tile per (head h, group g of 32 q-rows): psum [128=(4b x 32t), ext_g]
   ext_g = 256(g+1) (causal truncation at 64-granularity)
 - content scores: 4 matmuls (one per batch b) at partition bases 32b
 - rel scores: 8 accumulating block-diagonal matmuls; pass kp contracts
   k=(4 q-rows x 32 e) against the rel pack; the block-diag lhsT ("khat") is
   built from kq^T with strided copies
 - mask add, negated-max, fused exp+sum on ScalarE, normalize
 - P transposed 128x128 on TensorE, PV matmuls per (b, j-block) -> ctx^T
 - output projection from y^T with host-pretransposed Wo
"""
import os
import numpy as np

import concourse.bass as bass
import concourse.mybir as mybir
import concourse.tile as tile

F32 = mybir.dt.float32

B, T, C, H, E = 4, 1024, 256, 8, 32
NC = 8           # cores
TQ = T // NC     # 128 q rows per core
NG = 4           # row groups of 32 q rows
SCALE = 1.0 / np.sqrt(E)
NEG = -1.0e30


def ext_kp(g, kp):
    """rel pass width: pass kp of group g covers q-rows t_local in [4kp,4kp+4)."""
    return 256 * g + 64 * (kp // 2 + 1)


def ext_g(g):
    return 256 * (g + 1)


def chunks(ext):
    """split [0,ext) into <=512 col chunks (one psum bank each)"""
    if ext <= 512:
        return [(0, ext)]
    return [(0, 512), (512, ext)]


# packed rel column offsets: order (h, g, kp), contiguous per (h, g)
_PER_HG = [2048 * g + 1280 for g in range(NG)]       # sum_kp ext_kp(g, .)
_PER_H = sum(_PER_HG)                                # 17408
TOTCOL = H * _PER_H                                  # 139264


def _hg_off(h, g):
    return h * _PER_H + sum(_PER_HG[:g])



def _copy(nc, use_scalar, out, in_):
    if use_scalar:
        nc.scalar.copy(out, in_)
    else:
        nc.vector.tensor_copy(out, in_)


def sanitize_waits(nc):
    """This container's walrus accepts at most ONE sync wait per instruction.
    Hoist extra waits onto same-engine NOPs placed immediately before."""
    n = 0
    for f in nc.m.functions:
        for bb in f.blocks:
            new = []
            for inst in bb.instructions:
                si = inst.sync_info
                if si is not None and si.on_wait and len(si.on_wait) > 1:
                    waits = list(si.on_wait)
                    for w in waits[:-1]:
                        n += 1
                        nop = mybir.InstNoOp(
                            name=f"{inst.name}-sw{n}",
                            engine=inst.engine,
                            sync_info=mybir.SyncInfo(on_wait=[w], on_update=[]),
                            bass_nofuse=True,
                        )
                        new.append(nop)
                    si.on_wait = waits[-1:]
                new.append(inst)
            bb.instructions[:] = new
    return n


def build_program():
    nc = bass.Bass("TRN2")
    relp_d = nc.dram_tensor("relp", [128, TOTCOL], F32, kind="ExternalInput")
    xT_d = nc.dram_tensor("xT", [B, C, T], F32, kind="ExternalInput")
    xqT_d = nc.dram_tensor("xqT", [B, C, TQ], F32, kind="ExternalInput")
    WqT_d = nc.dram_tensor("WqT", [C, C], F32, kind="ExternalInput")
    WkT_d = nc.dram_tensor("WkT", [C, C], F32, kind="ExternalInput")
    WvT_d = nc.dram_tensor("WvT", [C, C], F32, kind="ExternalInput")
    WoT_d = nc.dram_tensor("WoT", [C, C], F32, kind="ExternalInput")
    msk_d = nc.dram_tensor("msk", [128, 256], F32, kind="ExternalInput")
    out_d = nc.dram_tensor("out", [B, TQ, C], F32, kind="ExternalOutput")

    with tile.TileContext(nc) as tc:
        with (
            tc.tile_pool(name="persist", bufs=1) as pp,
            tc.tile_pool(name="stream", bufs=2) as stp,
        ):
            # ---- persistent sbuf tensors ----
            ident = pp.tile([128, 128], F32, tag="ident", name="ident")
            from concourse.masks import make_identity
            make_identity(nc, ident[:])
            msk = pp.tile([128, 256], F32, tag="msk", name="msk")
            nc.sync.dma_start(msk[:], msk_d[:])
            w_sb = {}
            for nm, d in [("WqT", WqT_d), ("WkT", WkT_d), ("WvT", WvT_d), ("WoT", WoT_d)]:
                for half in range(2):
                    t_ = pp.tile([128, 256], F32, tag=f"{nm}{half}", name=f"{nm}{half}")
                    nc.sync.dma_start(t_[:], d[128 * half:128 * half + 128, :])
                    w_sb[(nm, half)] = t_
            KT = [pp.tile([128, B * T], F32, tag=f"KT{i}", name=f"KT{i}") for i in range(2)]
            QT = [pp.tile([128, B * TQ], F32, tag=f"QT{i}", name=f"QT{i}") for i in range(2)]
            kqT = [pp.tile([128, B * TQ], F32, tag=f"kqT{i}", name=f"kqT{i}") for i in range(2)]
            yT = [pp.tile([128, B * TQ], F32, tag=f"yT{i}", name=f"yT{i}") for i in range(2)]
            V = {}
            for b in range(B):
                for blk in range(8):
                    V[(b, blk)] = pp.tile([128, 256], F32, tag=f"V{b}_{blk}", name=f"V{b}_{blk}")

            # ================= prologue: projections =================
            with tc.tile_pool(name="prjps", bufs=2, space="PSUM") as prjps:
                for b in range(B):
                    xTb = [stp.tile([128, T], F32, tag=f"xTb{i}", name=f"xTb{i}") for i in range(2)]
                    for i in range(2):
                        nc.sync.dma_start(xTb[i][:], xT_d[b][128 * i:128 * i + 128, :])
                    xqTb = [stp.tile([128, TQ], F32, tag=f"xqTb{i}", name=f"xqTb{i}") for i in range(2)]
                    for i in range(2):
                        nc.sync.dma_start(xqTb[i][:], xqT_d[b][128 * i:128 * i + 128, :])
                    # K^T = Wk @ x^T  (per chat half, per 512-col chunk)
                    for ch in range(2):
                        for nk in range(2):
                            ps = prjps.tile([128, 512], F32, tag="prj", name="prj")
                            for kp in range(2):
                                nc.tensor.matmul(
                                    ps[:], w_sb[("WkT", kp)][:, 128 * ch:128 * ch + 128],
                                    xTb[kp][:, 512 * nk:512 * nk + 512],
                                    start=(kp == 0), stop=(kp == 1))
                            _copy(nc, (ch + nk) % 2, KT[ch][:, 1024 * b + 512 * nk:1024 * b + 512 * nk + 512], ps[:])
                        # Q^T and kq^T over the core's own q columns
                        for nm, dst in [("WqT", QT), ("WkT", kqT)]:
                            ps = prjps.tile([128, 512], F32, tag="prj", name="prj")
                            for kp in range(2):
                                nc.tensor.matmul(
                                    ps[:, 0:TQ], w_sb[(nm, kp)][:, 128 * ch:128 * ch + 128],
                                    xqTb[kp][:], start=(kp == 0), stop=(kp == 1))
                            nc.vector.tensor_copy(dst[ch][:, TQ * b:TQ * b + TQ], ps[:, 0:TQ])
                    # V = x @ Wv^T
                    for blk in range(8):
                        ps = prjps.tile([128, 512], F32, tag="prj", name="prj")
                        for kp in range(2):
                            nc.tensor.matmul(
                                ps[:, 0:256], xTb[kp][:, 128 * blk:128 * blk + 128],
                                w_sb[("WvT", kp)][:], start=(kp == 0), stop=(kp == 1))
                        _copy(nc, blk % 2, V[(b, blk)][:], ps[:, 0:256])

            # ================= main loop =================
            with (
                tc.tile_pool(name="rels", bufs=2) as relsp,
                tc.tile_pool(name="khp", bufs=2) as khp,
                tc.tile_pool(name="pp2", bufs=2) as pp2,
                tc.tile_pool(name="pts", bufs=3) as ptsp,
                tc.tile_pool(name="stats", bufs=3) as stats,
                tc.tile_pool(name="scps", bufs=2, space="PSUM") as scps,
                tc.tile_pool(name="ptps", bufs=2, space="PSUM") as ptps,
                tc.tile_pool(name="ctxps", bufs=2, space="PSUM") as ctxps,
            ):
                for h in range(H):
                    hh = h % 4
                    hi = h // 4
                    for g in range(NG):
                        eg = ext_g(g)
                        # --- load packed rel for this (h, g) ---
                        rels = relsp.tile([128, _PER_HG[g]], F32, tag="rels")
                        o0 = _hg_off(h, g)
                        nc.sync.dma_start(rels[:], relp_d[:, o0:o0 + _PER_HG[g]])
                        # --- build block-diag khat [128, 8*128] from kqT ---
                        kh = khp.tile([128, 1024], F32, tag="kh", name="kh")
                        nc.gpsimd.memset(kh[:], 0.0)
                        kq_src = kqT[hi]
                        for jtl in range(4):
                            dst_ap = bass.AP(kh[:].tensor, 32 * jtl * 1024 + jtl,
                                             [[1024, 32], [132, 8], [32, 4]])
                            src_ap = bass.AP(kq_src[:].tensor,
                                             (32 * hh) * (B * TQ) + 32 * g + jtl,
                                             [[B * TQ, 32], [4, 8], [TQ, 4]])
                            nc.vector.tensor_copy(dst_ap, src_ap)
                        # --- scores psum tile ---
                        SC = scps.tile([128, 1024], F32, tag="SC", name="SC")
                        # content first (start=True), bases 32b
                        for b in range(B):
                            lhsT = QT[hi][32 * hh:32 * hh + 32, TQ * b + 32 * g:TQ * b + 32 * g + 32]
                            for (c0, c1) in chunks(eg):
                                rhs = KT[hi][32 * hh:32 * hh + 32, T * b + c0:T * b + c1]
                                nc.tensor.matmul(SC[32 * b:32 * b + 32, c0:c1], lhsT, rhs,
                                                 start=True, stop=False,
                                                 tile_position=(32 * hh, 32 * b),
                                                 skip_group_check=True)
                        # rel passes, widest (kp=7) last carries stop
                        loc = 0
                        for kp in range(8):
                            ext = ext_kp(g, kp)
                            for (c0, c1) in chunks(ext):
                                nc.tensor.matmul(SC[:, c0:c1], kh[:, 128 * kp:128 * kp + 128],
                                                 rels[:, loc + c0:loc + c1],
                                                 start=False, stop=(kp == 7),
                                                 skip_group_check=True)
                            loc += ext
                        # --- mask (last 256 cols) ---
                        nc.vector.tensor_add(SC[:, 256 * g:256 * g + 256],
                                             SC[:, 256 * g:256 * g + 256], msk[:])
                        # --- softmax ---
                        nm = stats.tile([128, 1], F32, tag="nm", name="nm")
                        nc.vector.reduce_max(nm[:], SC[:, 0:eg], axis=mybir.AxisListType.X,
                                             negate=True)
                        nmb = stats.tile([128, 1], F32, tag="nmb", name="nmb")
                        nc.vector.tensor_scalar_mul(nmb[:], nm[:], SCALE)
                        P = pp2.tile([128, 1024], F32, tag="P", name="P")
                        sums = stats.tile([128, 1], F32, tag="sums", name="sums")
                        nc.scalar.activation(P[:, 0:eg], SC[:, 0:eg],
                                             mybir.ActivationFunctionType.Exp,
                                             bias=nmb[:], scale=SCALE, accum_out=sums[:])
                        rec = stats.tile([128, 1], F32, tag="rec", name="rec")
                        nc.vector.reciprocal(rec[:], sums[:])
                        nc.vector.tensor_scalar_mul(P[:, 0:eg], P[:, 0:eg], rec[:])
                        # --- PV: transpose P chunk, then 4 matmuls ---
                        ctx = ctxps.tile([128, 32], F32, tag="ctx", name="ctx")
                        njb = eg // 128
                        for jb in range(njb):
                            ptp = ptps.tile([128, 128], F32, tag="PTp", name="PTp")
                            nc.tensor.transpose(ptp[:], P[:, 128 * jb:128 * jb + 128], ident[:])
                            pts = ptsp.tile([128, 128], F32, tag="PTs", name="PTs")
                            _copy(nc, jb % 2, pts[:], ptp[:])
                            for b in range(B):
                                nc.tensor.matmul(ctx[32 * b:32 * b + 32, :],
                                                 V[(b, jb)][:, 32 * h:32 * h + 32],
                                                 pts[:, 32 * b:32 * b + 32],
                                                 start=(jb == 0), stop=(jb == njb - 1),
                                                 tile_position=(0, 32 * b),
                                                 skip_group_check=True)
                        # --- ctx^T -> y^T ---
                        for b in range(B):
                            _copy(nc, b % 2,
                                  yT[hi][32 * hh:32 * hh + 32, TQ * b + 32 * g:TQ * b + 32 * g + 32],
                                  ctx[32 * b:32 * b + 32, :])

                # ================= output projection =================
                for b in range(B):
                    ps = scps.tile([128, 256], F32, tag="SC", name="SC")
                    for half in range(2):
                        nc.tensor.matmul(ps[:], yT[half][:, TQ * b:TQ * b + TQ],
                                         w_sb[("WoT", half)][:],
                                         start=(half == 0), stop=(half == 1))
                    ot = pp2.tile([128, 256], F32, tag="P", name="P")
                    nc.vector.tensor_copy(ot[:], ps[:])
                    nc.sync.dma_start(out_d[b][:, :], ot[:])
    nsplit = sanitize_waits(nc)
    return nc


def pack_core(m, x, rel, mask_only=False):
    """Build per-core inputs: packed rel [128, TOTCOL], xqT, mask."""
    msk = np.zeros((128, 256), np.float32)
    jj = np.arange(256)[None, :]
    tl = (np.arange(128) % 32)[:, None]
    msk[jj > m + 8 * tl] = NEG
    if mask_only:
        return msk
    relp = np.empty((128, TOTCOL), np.float32)
    for g in range(NG):
        for kp in range(8):
            ext = ext_kp(g, kp)
            t0 = 32 * g + 4 * kp
            q0 = m + 8 * t0
            # rows q0, q0+8, q0+16, q0+24 ; block [(jtl,e), ext] for every head
            sl = rel[q0:q0 + 32:8, :ext, :, :]            # [4, ext, H, E]
            blk = sl.transpose(2, 0, 3, 1).reshape(H, 128, ext)  # [H, (jtl,e), ext]
            for h in range(H):
                o = _hg_off(h, g) + sum(ext_kp(g, k) for k in range(kp))
                relp[:, o:o + ext] = blk[h]
    xqT = np.ascontiguousarray(x[:, m::NC, :].transpose(0, 2, 1))
    return relp, xqT, msk


_CACHE = {}


def kernel(x, rel_encoding, Wq, Wk, Wv, Wo, unused=None, **_):
    x = np.asarray(x, np.float32)
    rel = np.asarray(rel_encoding, np.float32)
    if "nc" not in _CACHE:
        _CACHE["nc"] = build_program()
    nc = _CACHE["nc"]

    xT = np.ascontiguousarray(x.transpose(0, 2, 1))
    com = {
        "xT": xT,
        "WqT": np.ascontiguousarray(np.asarray(Wq, np.float32).T),
        "WkT": np.ascontiguousarray(np.asarray(Wk, np.float32).T),
        "WvT": np.ascontiguousarray(np.asarray(Wv, np.float32).T),
        "WoT": np.ascontiguousarray(np.asarray(Wo, np.float32).T),
    }
    in_maps = []
    for m in range(NC):
        relp, xqT, msk = pack_core(m, x, rel)
        im = dict(com)
        im.update({"relp": relp, "xqT": xqT, "msk": msk})
        in_maps.append(im)

    from concourse.bass_utils import run_bass_kernel_spmd
    res = run_bass_kernel_spmd(
        nc, in_maps, core_ids=list(range(NC)),
        trace=bool(int(os.environ.get("KERNEL_TRACE", "0"))),
    )
    _CACHE["last_results"] = res
    full = np.empty((B, T, C), np.float32)
    for m in range(NC):
        full[:, m::NC, :] = res.results[m]["out"]
    return full


# revision 7
# speedup vs baseline: 1.9924x; 1.9924x over previous
"""Causal self-attention with relative position encoding on 8 Trainium2 NeuronCores.

Problem: B=4, T=1024, C=256, H=8, E=32.
  q,k,v = x@W{q,k,v}.T ; att = q·k + einsum('qjhe,bhqe->bhqj', rel, k) ; scaled,
  causal-masked softmax ; y = att@v ; out = y@Wo.T

Sharding: query-row interleave across 8 cores (core m owns q = m+8t, t in [0,128)).
Each core loads a causally-trimmed, host-transposed pack of rel_encoding (~68 MiB
instead of 1 GiB), computes its 128 output rows fully, and the host re-interleaves.

Device-side structure per core (see block comments below):
 - scores tile per (head h, group g of 32 q-rows): psum [128=(4b x 32t), ext_g]
   ext_g = 256(g+1) (causal truncation at 64-granularity)
 - content scores: 4 matmuls (one per batch b) at partition bases 32b
 - rel scores: 8 accumulating block-diagonal matmuls; pass kp contracts
   k=(4 q-rows x 32 e) against the rel pack; the block-diag lhsT ("khat") is
   built from kq^T with strided copies
 - mask add, negated-max, fused exp+sum on ScalarE, normalize
 - P transposed 128x128 on TensorE, PV matmuls per (b, j-block) -> ctx^T
 - output projection from y^T with host-pretransposed Wo
"""
import os
import numpy as np

import concourse.bass as bass
import concourse.mybir as mybir
import concourse.tile as tile

F32 = mybir.dt.float32
BF16 = mybir.dt.bfloat16

B, T, C, H, E = 4, 1024, 256, 8, 32
NC = 8           # cores
TQ = T // NC     # 128 q rows per core
NG = 4           # row groups of 32 q rows
SCALE = 1.0 / np.sqrt(E)
NEG = -1.0e30


def ext_kp(g, kp):
    """rel pass width: pass kp of group g covers q-rows t_local in [4kp,4kp+4)."""
    return 256 * g + 64 * (kp // 2 + 1)


def ext_g(g):
    return 256 * (g + 1)


def chunks(ext):
    """split [0,ext) into <=512 col chunks (one psum bank each).
    Matmul out must stay within one 2KB psum bank -> 512 f32 cols."""
    if ext <= 512:
        return [(0, ext)]
    return [(0, 512), (512, ext)]


# packed rel column offsets: order (h, g, kp), contiguous per (h, g)
_PER_HG = [2048 * g + 1280 for g in range(NG)]       # sum_kp ext_kp(g, .)
_PER_H = sum(_PER_HG)                                # 17408
TOTCOL = H * _PER_H                                  # 139264


def _hg_off(h, g):
    return h * _PER_H + sum(_PER_HG[:g])



def _copy(nc, use_scalar, out, in_):
    if use_scalar:
        nc.scalar.copy(out, in_)
    else:
        nc.vector.tensor_copy(out, in_)


def sanitize_waits(nc):
    """This container's walrus accepts at most ONE sync wait per instruction.
    Hoist extra waits onto same-engine NOPs placed immediately before."""
    n = 0
    for f in nc.m.functions:
        for bb in f.blocks:
            new = []
            for inst in bb.instructions:
                si = inst.sync_info
                if si is not None and si.on_wait and len(si.on_wait) > 1:
                    waits = list(si.on_wait)
                    for w in waits[:-1]:
                        n += 1
                        nop = mybir.InstNoOp(
                            name=f"{inst.name}-sw{n}",
                            engine=inst.engine,
                            sync_info=mybir.SyncInfo(on_wait=[w], on_update=[]),
                            bass_nofuse=True,
                        )
                        new.append(nop)
                    si.on_wait = waits[-1:]
                new.append(inst)
            bb.instructions[:] = new
    return n


def build_program():
    nc = bass.Bass("TRN2")
    relp_d = nc.dram_tensor("relp", [128, TOTCOL], BF16, kind="ExternalInput")
    xT_d = nc.dram_tensor("xT", [B, C, T], F32, kind="ExternalInput")
    xqT_d = nc.dram_tensor("xqT", [B, C, TQ], F32, kind="ExternalInput")
    WqT_d = nc.dram_tensor("WqT", [C, C], F32, kind="ExternalInput")
    WkT_d = nc.dram_tensor("WkT", [C, C], F32, kind="ExternalInput")
    WvT_d = nc.dram_tensor("WvT", [C, C], F32, kind="ExternalInput")
    WoT_d = nc.dram_tensor("WoT", [C, C], F32, kind="ExternalInput")
    msk_d = nc.dram_tensor("msk", [128, 256], F32, kind="ExternalInput")
    out_d = nc.dram_tensor("out", [B, TQ, C], F32, kind="ExternalOutput")

    with tile.TileContext(nc) as tc:
        with (
            tc.tile_pool(name="persist", bufs=1) as pp,
            tc.tile_pool(name="stream", bufs=2) as stp,
        ):
            # ---- persistent sbuf tensors ----
            ident = pp.tile([128, 128], BF16, tag="ident", name="ident")
            from concourse.masks import make_identity
            make_identity(nc, ident[:])
            msk = pp.tile([128, 256], F32, tag="msk", name="msk")
            nc.sync.dma_start(msk[:], msk_d[:])
            w_sb = {}
            for nm, d in [("WqT", WqT_d), ("WkT", WkT_d), ("WvT", WvT_d), ("WoT", WoT_d)]:
                for half in range(2):
                    t_ = pp.tile([128, 256], F32, tag=f"{nm}{half}", name=f"{nm}{half}")
                    nc.sync.dma_start(t_[:], d[128 * half:128 * half + 128, :])
                    w_sb[(nm, half)] = t_
            KT = [pp.tile([128, B * T], BF16, tag=f"KT{i}", name=f"KT{i}") for i in range(2)]
            QT = [pp.tile([128, B * TQ], BF16, tag=f"QT{i}", name=f"QT{i}") for i in range(2)]
            kqT = [pp.tile([128, B * TQ], BF16, tag=f"kqT{i}", name=f"kqT{i}") for i in range(2)]
            yT = [pp.tile([128, B * TQ], F32, tag=f"yT{i}", name=f"yT{i}") for i in range(2)]
            V = {}
            for b in range(B):
                for blk in range(8):
                    V[(b, blk)] = pp.tile([128, 256], BF16, tag=f"V{b}_{blk}", name=f"V{b}_{blk}")

            # ================= prologue: projections =================
            with tc.tile_pool(name="prjps", bufs=2, space="PSUM") as prjps:
                for b in range(B):
                    xTb = [stp.tile([128, T], F32, tag=f"xTb{i}", name=f"xTb{i}") for i in range(2)]
                    for i in range(2):
                        nc.sync.dma_start(xTb[i][:], xT_d[b][128 * i:128 * i + 128, :])
                    xqTb = [stp.tile([128, TQ], F32, tag=f"xqTb{i}", name=f"xqTb{i}") for i in range(2)]
                    for i in range(2):
                        nc.sync.dma_start(xqTb[i][:], xqT_d[b][128 * i:128 * i + 128, :])
                    # K^T = Wk @ x^T  (per chat half, per 512-col chunk)
                    for ch in range(2):
                        for nk in range(2):
                            ps = prjps.tile([128, 512], F32, tag="prj", name="prj")
                            for kp in range(2):
                                nc.tensor.matmul(
                                    ps[:], w_sb[("WkT", kp)][:, 128 * ch:128 * ch + 128],
                                    xTb[kp][:, 512 * nk:512 * nk + 512],
                                    start=(kp == 0), stop=(kp == 1))
                            _copy(nc, (ch + nk) % 2, KT[ch][:, 1024 * b + 512 * nk:1024 * b + 512 * nk + 512], ps[:])
                        # Q^T and kq^T over the core's own q columns
                        for nm, dst in [("WqT", QT), ("WkT", kqT)]:
                            ps = prjps.tile([128, 512], F32, tag="prj", name="prj")
                            for kp in range(2):
                                nc.tensor.matmul(
                                    ps[:, 0:TQ], w_sb[(nm, kp)][:, 128 * ch:128 * ch + 128],
                                    xqTb[kp][:], start=(kp == 0), stop=(kp == 1))
                            nc.vector.tensor_copy(dst[ch][:, TQ * b:TQ * b + TQ], ps[:, 0:TQ])
                    # V = x @ Wv^T
                    for blk in range(8):
                        ps = prjps.tile([128, 512], F32, tag="prj", name="prj")
                        for kp in range(2):
                            nc.tensor.matmul(
                                ps[:, 0:256], xTb[kp][:, 128 * blk:128 * blk + 128],
                                w_sb[("WvT", kp)][:], start=(kp == 0), stop=(kp == 1))
                        _copy(nc, blk % 2, V[(b, blk)][:], ps[:, 0:256])

            # ================= main loop =================
            with (
                tc.tile_pool(name="rels", bufs=2) as relsp,
                tc.tile_pool(name="khp", bufs=2) as khp,
                tc.tile_pool(name="pp2", bufs=2) as pp2,
                tc.tile_pool(name="pts", bufs=3) as ptsp,
                tc.tile_pool(name="stats", bufs=3) as stats,
                tc.tile_pool(name="scps", bufs=2, space="PSUM") as scps,
                tc.tile_pool(name="ptps", bufs=2, space="PSUM") as ptps,
                tc.tile_pool(name="ctxps", bufs=2, space="PSUM") as ctxps,
            ):
                for h in range(H):
                    hh = h % 4
                    hi = h // 4
                    for g in range(NG):
                        eg = ext_g(g)
                        # --- load packed rel for this (h, g) ---
                        rels = relsp.tile([128, _PER_HG[g]], BF16, tag="rels", name="rels")
                        o0 = _hg_off(h, g)
                        nc.sync.dma_start(rels[:], relp_d[:, o0:o0 + _PER_HG[g]])
                        # --- build block-diag khat [128, 8*128] from kqT ---
                        kh = khp.tile([128, 1024], BF16, tag="kh", name="kh")
                        nc.gpsimd.memset(kh[:], 0.0)
                        kq_src = kqT[hi]
                        for jtl in range(4):
                            dst_ap = bass.AP(kh[:].tensor, 32 * jtl * 1024 + jtl,
                                             [[1024, 32], [132, 8], [32, 4]])
                            src_ap = bass.AP(kq_src[:].tensor,
                                             (32 * hh) * (B * TQ) + 32 * g + jtl,
                                             [[B * TQ, 32], [4, 8], [TQ, 4]])
                            nc.vector.tensor_copy(dst_ap, src_ap)
                        # --- scores psum tile ---
                        SC = scps.tile([128, 1024], F32, tag="SC", name="SC")
                        # content first (start=True), bases 32b
                        for b in range(B):
                            lhsT = QT[hi][32 * hh:32 * hh + 32, TQ * b + 32 * g:TQ * b + 32 * g + 32]
                            for (c0, c1) in chunks(eg):
                                rhs = KT[hi][32 * hh:32 * hh + 32, T * b + c0:T * b + c1]
                                nc.tensor.matmul(SC[32 * b:32 * b + 32, c0:c1], lhsT, rhs,
                                                 start=True, stop=False,
                                                 tile_position=(32 * hh, 32 * b),
                                                 skip_group_check=True)
                        # rel passes, widest (kp=7) last carries stop
                        loc = 0
                        for kp in range(8):
                            ext = ext_kp(g, kp)
                            for (c0, c1) in chunks(ext):
                                nc.tensor.matmul(SC[:, c0:c1], kh[:, 128 * kp:128 * kp + 128],
                                                 rels[:, loc + c0:loc + c1],
                                                 start=False, stop=(kp == 7),
                                                 skip_group_check=True)
                            loc += ext
                        # --- mask (last 256 cols) ---
                        nc.vector.tensor_add(SC[:, 256 * g:256 * g + 256],
                                             SC[:, 256 * g:256 * g + 256], msk[:])
                        # --- softmax ---
                        nm = stats.tile([128, 1], F32, tag="nm", name="nm")
                        nc.vector.reduce_max(nm[:], SC[:, 0:eg], axis=mybir.AxisListType.X,
                                             negate=True)
                        nmb = stats.tile([128, 1], F32, tag="nmb", name="nmb")
                        nc.vector.tensor_scalar_mul(nmb[:], nm[:], SCALE)
                        P = pp2.tile([128, 1024], BF16, tag="P", name="P")
                        sums = stats.tile([128, 1], F32, tag="sums", name="sums")
                        nc.scalar.activation(P[:, 0:eg], SC[:, 0:eg],
                                             mybir.ActivationFunctionType.Exp,
                                             bias=nmb[:], scale=SCALE, accum_out=sums[:])
                        rec = stats.tile([128, 1], F32, tag="rec", name="rec")
                        nc.vector.reciprocal(rec[:], sums[:])
                        nc.vector.tensor_scalar_mul(P[:, 0:eg], P[:, 0:eg], rec[:])
                        # --- PV: transpose P chunk, then 4 matmuls ---
                        ctx = ctxps.tile([128, 32], F32, tag="ctx", name="ctx")
                        njb = eg // 128
                        for jb in range(njb):
                            ptp = ptps.tile([128, 128], BF16, tag="PTp", name="PTp")
                            nc.tensor.transpose(ptp[:], P[:, 128 * jb:128 * jb + 128], ident[:])
                            pts = ptsp.tile([128, 128], BF16, tag="PTs", name="PTs")
                            _copy(nc, jb % 2, pts[:], ptp[:])
                            for b in range(B):
                                nc.tensor.matmul(ctx[32 * b:32 * b + 32, :],
                                                 V[(b, jb)][:, 32 * h:32 * h + 32],
                                                 pts[:, 32 * b:32 * b + 32],
                                                 start=(jb == 0), stop=(jb == njb - 1),
                                                 tile_position=(0, 32 * b),
                                                 skip_group_check=True)
                        # --- ctx^T -> y^T ---
                        for b in range(B):
                            _copy(nc, b % 2,
                                  yT[hi][32 * hh:32 * hh + 32, TQ * b + 32 * g:TQ * b + 32 * g + 32],
                                  ctx[32 * b:32 * b + 32, :])

                # ================= output projection =================
                for b in range(B):
                    ps = scps.tile([128, 256], F32, tag="SC", name="SC")
                    for half in range(2):
                        nc.tensor.matmul(ps[:], yT[half][:, TQ * b:TQ * b + TQ],
                                         w_sb[("WoT", half)][:],
                                         start=(half == 0), stop=(half == 1))
                    ot = pp2.tile([128, 256], F32, tag="oex", name="oex")
                    nc.vector.tensor_copy(ot[:], ps[:])
                    nc.sync.dma_start(out_d[b][:, :], ot[:])
    nsplit = sanitize_waits(nc)
    return nc


def pack_core(m, x, rel, mask_only=False):
    """Build per-core inputs: packed rel [128, TOTCOL], xqT, mask."""
    msk = np.zeros((128, 256), np.float32)
    jj = np.arange(256)[None, :]
    tl = (np.arange(128) % 32)[:, None]
    msk[jj > m + 8 * tl] = NEG
    if mask_only:
        return msk
    import ml_dtypes
    relp = np.empty((128, TOTCOL), ml_dtypes.bfloat16)
    for g in range(NG):
        for kp in range(8):
            ext = ext_kp(g, kp)
            t0 = 32 * g + 4 * kp
            q0 = m + 8 * t0
            # rows q0, q0+8, q0+16, q0+24 ; block [(jtl,e), ext] for every head
            sl = rel[q0:q0 + 32:8, :ext, :, :]            # [4, ext, H, E]
            blk = sl.transpose(2, 0, 3, 1).reshape(H, 128, ext)  # [H, (jtl,e), ext]
            for h in range(H):
                o = _hg_off(h, g) + sum(ext_kp(g, k) for k in range(kp))
                relp[:, o:o + ext] = blk[h]
    xqT = np.ascontiguousarray(x[:, m::NC, :].transpose(0, 2, 1))
    return relp, xqT, msk


_CACHE = {}


def kernel(x, rel_encoding, Wq, Wk, Wv, Wo, unused=None, **_):
    x = np.asarray(x, np.float32)
    rel = np.asarray(rel_encoding, np.float32)
    if "nc" not in _CACHE:
        _CACHE["nc"] = build_program()
    nc = _CACHE["nc"]

    xT = np.ascontiguousarray(x.transpose(0, 2, 1))
    com = {
        "xT": xT,
        "WqT": np.ascontiguousarray(np.asarray(Wq, np.float32).T),
        "WkT": np.ascontiguousarray(np.asarray(Wk, np.float32).T),
        "WvT": np.ascontiguousarray(np.asarray(Wv, np.float32).T),
        "WoT": np.ascontiguousarray(np.asarray(Wo, np.float32).T),
    }
    in_maps = []
    for m in range(NC):
        relp, xqT, msk = pack_core(m, x, rel)
        im = dict(com)
        im.update({"relp": relp, "xqT": xqT, "msk": msk})
        in_maps.append(im)

    from concourse.bass_utils import run_bass_kernel_spmd
    res = run_bass_kernel_spmd(
        nc, in_maps, core_ids=list(range(NC)),
        trace=bool(int(os.environ.get("KERNEL_TRACE", "0"))),
    )
    _CACHE["last_results"] = res
    full = np.empty((B, T, C), np.float32)
    for m in range(NC):
        full[:, m::NC, :] = res.results[m]["out"]
    return full


# revision 8
# speedup vs baseline: 2.1145x; 1.0613x over previous
"""Causal self-attention with relative position encoding on 8 Trainium2 NeuronCores.

Problem: B=4, T=1024, C=256, H=8, E=32.
  q,k,v = x@W{q,k,v}.T ; att = q·k + einsum('qjhe,bhqe->bhqj', rel, k) ; scaled,
  causal-masked softmax ; y = att@v ; out = y@Wo.T

Sharding: query-row interleave across 8 cores (core m owns q = m+8t, t in [0,128)).
Each core loads a causally-trimmed, host-transposed pack of rel_encoding (~68 MiB
instead of 1 GiB), computes its 128 output rows fully, and the host re-interleaves.

Device-side structure per core (see block comments below):
 - scores tile per (head h, group g of 32 q-rows): psum [128=(4b x 32t), ext_g]
   ext_g = 256(g+1) (causal truncation at 64-granularity)
 - content scores: 4 matmuls (one per batch b) at partition bases 32b
 - rel scores: 8 accumulating block-diagonal matmuls; pass kp contracts
   k=(4 q-rows x 32 e) against the rel pack; the block-diag lhsT ("khat") is
   built from kq^T with strided copies
 - mask add, negated-max, fused exp+sum on ScalarE, normalize
 - P transposed 128x128 on TensorE, PV matmuls per (b, j-block) -> ctx^T
 - output projection from y^T with host-pretransposed Wo
"""
import os
import numpy as np

import concourse.bass as bass
import concourse.mybir as mybir
import concourse.tile as tile

F32 = mybir.dt.float32
BF16 = mybir.dt.bfloat16

B, T, C, H, E = 4, 1024, 256, 8, 32
NC = 8           # cores
TQ = T // NC     # 128 q rows per core
NG = 4           # row groups of 32 q rows
SCALE = 1.0 / np.sqrt(E)
NEG = -1.0e30


def ext_kp(g, kp):
    """rel pass width: pass kp of group g covers q-rows t_local in [4kp,4kp+4)."""
    return 256 * g + 64 * (kp // 2 + 1)


def ext_g(g):
    return 256 * (g + 1)


def chunks(ext):
    """split [0,ext) into <=512 col chunks (one psum bank each).
    Matmul out must stay within one 2KB psum bank -> 512 f32 cols."""
    if ext <= 512:
        return [(0, ext)]
    return [(0, 512), (512, ext)]


# packed rel column offsets: order (h, g, kp), contiguous per (h, g)
_PER_HG = [2048 * g + 1280 for g in range(NG)]       # sum_kp ext_kp(g, .)
_PER_H = sum(_PER_HG)                                # 17408
TOTCOL = H * _PER_H                                  # 139264


def _hg_off(h, g):
    return h * _PER_H + sum(_PER_HG[:g])



def _copy(nc, use_scalar, out, in_):
    if use_scalar:
        nc.scalar.copy(out, in_)
    else:
        nc.vector.tensor_copy(out, in_)


def sanitize_waits(nc):
    """This container's walrus accepts at most ONE sync wait per instruction.
    Hoist extra waits onto same-engine NOPs placed immediately before."""
    n = 0
    for f in nc.m.functions:
        for bb in f.blocks:
            new = []
            for inst in bb.instructions:
                si = inst.sync_info
                if si is not None and si.on_wait and len(si.on_wait) > 1:
                    waits = list(si.on_wait)
                    for w in waits[:-1]:
                        n += 1
                        nop = mybir.InstNoOp(
                            name=f"{inst.name}-sw{n}",
                            engine=inst.engine,
                            sync_info=mybir.SyncInfo(on_wait=[w], on_update=[]),
                            bass_nofuse=True,
                        )
                        new.append(nop)
                    si.on_wait = waits[-1:]
                new.append(inst)
            bb.instructions[:] = new
    return n


def build_program():
    nc = bass.Bass("TRN2")
    relp_d = nc.dram_tensor("relp", [128, TOTCOL], BF16, kind="ExternalInput")
    xT_d = nc.dram_tensor("xT", [B, C, T], BF16, kind="ExternalInput")
    xqT_d = nc.dram_tensor("xqT", [B, C, TQ], BF16, kind="ExternalInput")
    WqT_d = nc.dram_tensor("WqT", [C, C], BF16, kind="ExternalInput")
    WkT_d = nc.dram_tensor("WkT", [C, C], BF16, kind="ExternalInput")
    WvT_d = nc.dram_tensor("WvT", [C, C], BF16, kind="ExternalInput")
    WoT_d = nc.dram_tensor("WoT", [C, C], F32, kind="ExternalInput")
    msk_d = nc.dram_tensor("msk", [128, 256], F32, kind="ExternalInput")
    out_d = nc.dram_tensor("out", [B, TQ, C], F32, kind="ExternalOutput")

    with tile.TileContext(nc) as tc:
        with (
            tc.tile_pool(name="persist", bufs=1) as pp,
            tc.tile_pool(name="stream", bufs=2) as stp,
        ):
            # ---- persistent sbuf tensors ----
            ident = pp.tile([128, 128], BF16, tag="ident", name="ident")
            from concourse.masks import make_identity
            make_identity(nc, ident[:])
            msk = pp.tile([128, 256], F32, tag="msk", name="msk")
            nc.sync.dma_start(msk[:], msk_d[:])
            w_sb = {}
            for nm, d in [("WqT", WqT_d), ("WkT", WkT_d), ("WvT", WvT_d), ("WoT", WoT_d)]:
                for half in range(2):
                    wdt = F32 if nm == "WoT" else BF16
                    t_ = pp.tile([128, 256], wdt, tag=f"{nm}{half}", name=f"{nm}{half}")
                    nc.sync.dma_start(t_[:], d[128 * half:128 * half + 128, :])
                    w_sb[(nm, half)] = t_
            KT = [pp.tile([128, B * T], BF16, tag=f"KT{i}", name=f"KT{i}") for i in range(2)]
            QT = [pp.tile([128, B * TQ], BF16, tag=f"QT{i}", name=f"QT{i}") for i in range(2)]
            kqT = [pp.tile([128, B * TQ], BF16, tag=f"kqT{i}", name=f"kqT{i}") for i in range(2)]
            yT = [pp.tile([128, B * TQ], F32, tag=f"yT{i}", name=f"yT{i}") for i in range(2)]
            V = {}
            for b in range(B):
                for blk in range(8):
                    V[(b, blk)] = pp.tile([128, 256], BF16, tag=f"V{b}_{blk}", name=f"V{b}_{blk}")

            # ================= prologue: projections =================
            with tc.tile_pool(name="prjps", bufs=2, space="PSUM") as prjps:
                for b in range(B):
                    xTb = [stp.tile([128, T], BF16, tag=f"xTb{i}", name=f"xTb{i}") for i in range(2)]
                    for i in range(2):
                        nc.sync.dma_start(xTb[i][:], xT_d[b][128 * i:128 * i + 128, :])
                    xqTb = [stp.tile([128, TQ], BF16, tag=f"xqTb{i}", name=f"xqTb{i}") for i in range(2)]
                    for i in range(2):
                        nc.sync.dma_start(xqTb[i][:], xqT_d[b][128 * i:128 * i + 128, :])
                    # K^T = Wk @ x^T  (per chat half, per 512-col chunk)
                    for ch in range(2):
                        for nk in range(2):
                            ps = prjps.tile([128, 512], F32, tag="prj", name="prj")
                            for kp in range(2):
                                nc.tensor.matmul(
                                    ps[:], w_sb[("WkT", kp)][:, 128 * ch:128 * ch + 128],
                                    xTb[kp][:, 512 * nk:512 * nk + 512],
                                    start=(kp == 0), stop=(kp == 1))
                            _copy(nc, (ch + nk) % 2, KT[ch][:, 1024 * b + 512 * nk:1024 * b + 512 * nk + 512], ps[:])
                        # Q^T and kq^T over the core's own q columns
                        for nm, dst in [("WqT", QT), ("WkT", kqT)]:
                            ps = prjps.tile([128, 512], F32, tag="prj", name="prj")
                            for kp in range(2):
                                nc.tensor.matmul(
                                    ps[:, 0:TQ], w_sb[(nm, kp)][:, 128 * ch:128 * ch + 128],
                                    xqTb[kp][:], start=(kp == 0), stop=(kp == 1))
                            nc.vector.tensor_copy(dst[ch][:, TQ * b:TQ * b + TQ], ps[:, 0:TQ])
                    # V = x @ Wv^T
                    for blk in range(8):
                        ps = prjps.tile([128, 512], F32, tag="prj", name="prj")
                        for kp in range(2):
                            nc.tensor.matmul(
                                ps[:, 0:256], xTb[kp][:, 128 * blk:128 * blk + 128],
                                w_sb[("WvT", kp)][:], start=(kp == 0), stop=(kp == 1))
                        _copy(nc, blk % 2, V[(b, blk)][:], ps[:, 0:256])

            # ================= main loop =================
            with (
                tc.tile_pool(name="rels", bufs=2) as relsp,
                tc.tile_pool(name="khp", bufs=2) as khp,
                tc.tile_pool(name="pp2", bufs=2) as pp2,
                tc.tile_pool(name="pts", bufs=3) as ptsp,
                tc.tile_pool(name="stats", bufs=3) as stats,
                tc.tile_pool(name="scps", bufs=2, space="PSUM") as scps,
                tc.tile_pool(name="ptps", bufs=2, space="PSUM") as ptps,
                tc.tile_pool(name="ctxps", bufs=2, space="PSUM") as ctxps,
            ):
                for h in range(H):
                    hh = h % 4
                    hi = h // 4
                    for g in range(NG):
                        eg = ext_g(g)
                        # --- load packed rel for this (h, g) ---
                        rels = relsp.tile([128, _PER_HG[g]], BF16, tag="rels", name="rels")
                        o0 = _hg_off(h, g)
                        nc.sync.dma_start(rels[:], relp_d[:, o0:o0 + _PER_HG[g]])
                        # --- build block-diag khat [128, 8*128] from kqT ---
                        kh = khp.tile([128, 1024], BF16, tag="kh", name="kh")
                        nc.gpsimd.memset(kh[:], 0.0)
                        kq_src = kqT[hi]
                        for jtl in range(4):
                            dst_ap = bass.AP(kh[:].tensor, 32 * jtl * 1024 + jtl,
                                             [[1024, 32], [132, 8], [32, 4]])
                            src_ap = bass.AP(kq_src[:].tensor,
                                             (32 * hh) * (B * TQ) + 32 * g + jtl,
                                             [[B * TQ, 32], [4, 8], [TQ, 4]])
                            nc.vector.tensor_copy(dst_ap, src_ap)
                        # --- scores psum tile ---
                        SC = scps.tile([128, 1024], F32, tag="SC", name="SC")
                        # content first (start=True), bases 32b
                        for b in range(B):
                            lhsT = QT[hi][32 * hh:32 * hh + 32, TQ * b + 32 * g:TQ * b + 32 * g + 32]
                            for (c0, c1) in chunks(eg):
                                rhs = KT[hi][32 * hh:32 * hh + 32, T * b + c0:T * b + c1]
                                nc.tensor.matmul(SC[32 * b:32 * b + 32, c0:c1], lhsT, rhs,
                                                 start=True, stop=False,
                                                 tile_position=(32 * hh, 32 * b),
                                                 skip_group_check=True)
                        # rel passes, widest (kp=7) last carries stop
                        loc = 0
                        for kp in range(8):
                            ext = ext_kp(g, kp)
                            for (c0, c1) in chunks(ext):
                                nc.tensor.matmul(SC[:, c0:c1], kh[:, 128 * kp:128 * kp + 128],
                                                 rels[:, loc + c0:loc + c1],
                                                 start=False, stop=(kp == 7),
                                                 skip_group_check=True)
                            loc += ext
                        # --- mask (last 256 cols) ---
                        nc.vector.tensor_add(SC[:, 256 * g:256 * g + 256],
                                             SC[:, 256 * g:256 * g + 256], msk[:])
                        # --- softmax ---
                        nm = stats.tile([128, 1], F32, tag="nm", name="nm")
                        nc.vector.reduce_max(nm[:], SC[:, 0:eg], axis=mybir.AxisListType.X,
                                             negate=True)
                        nmb = stats.tile([128, 1], F32, tag="nmb", name="nmb")
                        nc.vector.tensor_scalar_mul(nmb[:], nm[:], SCALE)
                        P = pp2.tile([128, 1024], BF16, tag="P", name="P")
                        sums = stats.tile([128, 1], F32, tag="sums", name="sums")
                        nc.scalar.activation(P[:, 0:eg], SC[:, 0:eg],
                                             mybir.ActivationFunctionType.Exp,
                                             bias=nmb[:], scale=SCALE, accum_out=sums[:])
                        rec = stats.tile([128, 1], F32, tag="rec", name="rec")
                        nc.vector.reciprocal(rec[:], sums[:])
                        nc.vector.tensor_scalar_mul(P[:, 0:eg], P[:, 0:eg], rec[:])
                        # --- PV: transpose P chunk, then 4 matmuls ---
                        ctx = ctxps.tile([128, 32], F32, tag="ctx", name="ctx")
                        njb = eg // 128
                        for jb in range(njb):
                            ptp = ptps.tile([128, 128], BF16, tag="PTp", name="PTp")
                            nc.tensor.transpose(ptp[:], P[:, 128 * jb:128 * jb + 128], ident[:])
                            pts = ptsp.tile([128, 128], BF16, tag="PTs", name="PTs")
                            _copy(nc, jb % 2, pts[:], ptp[:])
                            for b in range(B):
                                nc.tensor.matmul(ctx[32 * b:32 * b + 32, :],
                                                 V[(b, jb)][:, 32 * h:32 * h + 32],
                                                 pts[:, 32 * b:32 * b + 32],
                                                 start=(jb == 0), stop=(jb == njb - 1),
                                                 tile_position=(0, 32 * b),
                                                 skip_group_check=True)
                        # --- ctx^T -> y^T ---
                        for b in range(B):
                            _copy(nc, b % 2,
                                  yT[hi][32 * hh:32 * hh + 32, TQ * b + 32 * g:TQ * b + 32 * g + 32],
                                  ctx[32 * b:32 * b + 32, :])

                # ================= output projection =================
                for b in range(B):
                    ps = scps.tile([128, 256], F32, tag="SC", name="SC")
                    for half in range(2):
                        nc.tensor.matmul(ps[:], yT[half][:, TQ * b:TQ * b + TQ],
                                         w_sb[("WoT", half)][:],
                                         start=(half == 0), stop=(half == 1))
                    ot = pp2.tile([128, 256], F32, tag="oex", name="oex")
                    nc.vector.tensor_copy(ot[:], ps[:])
                    nc.sync.dma_start(out_d[b][:, :], ot[:])
    nsplit = sanitize_waits(nc)
    return nc


def pack_core(m, x, rel, mask_only=False):
    """Build per-core inputs: packed rel [128, TOTCOL], xqT, mask."""
    msk = np.zeros((128, 256), np.float32)
    jj = np.arange(256)[None, :]
    tl = (np.arange(128) % 32)[:, None]
    msk[jj > m + 8 * tl] = NEG
    if mask_only:
        return msk
    import ml_dtypes
    relp = np.empty((128, TOTCOL), ml_dtypes.bfloat16)
    for g in range(NG):
        for kp in range(8):
            ext = ext_kp(g, kp)
            t0 = 32 * g + 4 * kp
            q0 = m + 8 * t0
            # rows q0, q0+8, q0+16, q0+24 ; block [(jtl,e), ext] for every head
            sl = rel[q0:q0 + 32:8, :ext, :, :]            # [4, ext, H, E]
            blk = sl.transpose(2, 0, 3, 1).reshape(H, 128, ext)  # [H, (jtl,e), ext]
            for h in range(H):
                o = _hg_off(h, g) + sum(ext_kp(g, k) for k in range(kp))
                relp[:, o:o + ext] = blk[h]
    import ml_dtypes as _md
    xqT = np.ascontiguousarray(x[:, m::NC, :].transpose(0, 2, 1)).astype(_md.bfloat16)
    return relp, xqT, msk


_CACHE = {}


def kernel(x, rel_encoding, Wq, Wk, Wv, Wo, unused=None, **_):
    x = np.asarray(x, np.float32)
    rel = np.asarray(rel_encoding, np.float32)
    if "nc" not in _CACHE:
        _CACHE["nc"] = build_program()
    nc = _CACHE["nc"]

    import ml_dtypes
    xT = np.ascontiguousarray(x.transpose(0, 2, 1)).astype(ml_dtypes.bfloat16)
    com = {
        "xT": xT,
        "WqT": np.ascontiguousarray(np.asarray(Wq, np.float32).T).astype(ml_dtypes.bfloat16),
        "WkT": np.ascontiguousarray(np.asarray(Wk, np.float32).T).astype(ml_dtypes.bfloat16),
        "WvT": np.ascontiguousarray(np.asarray(Wv, np.float32).T).astype(ml_dtypes.bfloat16),
        "WoT": np.ascontiguousarray(np.asarray(Wo, np.float32).T),
    }
    in_maps = []
    for m in range(NC):
        relp, xqT, msk = pack_core(m, x, rel)
        im = dict(com)
        im.update({"relp": relp, "xqT": xqT, "msk": msk})
        in_maps.append(im)

    from concourse.bass_utils import run_bass_kernel_spmd
    res = run_bass_kernel_spmd(
        nc, in_maps, core_ids=list(range(NC)),
        trace=bool(int(os.environ.get("KERNEL_TRACE", "0"))),
    )
    _CACHE["last_results"] = res
    full = np.empty((B, T, C), np.float32)
    for m in range(NC):
        full[:, m::NC, :] = res.results[m]["out"]
    return full
